# revision 1
# baseline (speedup 1.0000x reference)
"""Trainium2 Bass/Tile kernel for nn_Attention_50242527428847.

Computation (per batch element b):
    dec[t,e]   = sum_h decoder_states[t,b,h] * W[e,h]            (projection)
    p[t,s,e]   = exp(dec[t,e] * encoder_states[s,b,e])           (softmax numerator over s)
    denom[t,e] = sum_s p[t,s,e]
    wsum[t,s]  = sum_e p[t,s,e] / denom[t,e]
    out[t,b,d] = sum_s wsum[t,s] * encoder_inputs[s,b,d]

Key algebraic points:
  - The reference einsum "tsbe,sbd->tbd" contracts BOTH s and e, so only
    wsum (128x128 per b) is ever needed, never the 4D weight tensor, and the
    1/denom weighting rides the e-contraction as the matmul's moving operand.
  - No max-subtraction: scores are products of ~N(0,1) values, far inside
    fp32 exp range.

Sharding: batch dim B=8, one batch element per NeuronCore (data parallel).
All transposes (W.T per e-chunk, decoder.T, encoder.T) are host-side numpy.

Per-core layout: p[e_local(128 part), e_chunk(4), t_local, s], processed in
NBLK t-blocks of TB, software-pipelined by the Tile scheduler (p bufs=4).

Engine split, balanced to ~85us each on ACT/DVE/GPSIMD (cost model):
  - ACT:  all exps.  Two "fused" blocks compute p = exp(enc_T * scale=dec_col)
          directly with per-partition scale APs (one instr per (t, e-chunk),
          absorbing the multiply); other blocks use large-FD plain exp,
          in-place over the scores.
  - GPSIMD: broadcast multiplies (stride-0 APs) for most (block, chunk)s,
          plus halving-tree reduces for the tail blocks' denominators.
  - DVE:  remaining multiplies, segmented reduce_sum over s (denominators),
          reciprocal, PSUM->SBUF copies.
  - PE:   projection matmuls; per-(t, e-chunk) N=1 matmuls
          (lhsT=p-chunk [K=e,M=s], rhs=1/denom column) accumulating
          wsum_T[s,t] columns in PSUM; final out = wsum_T.T @ enc_in done per
          block-pair (M=32, tile_position for partition offsets).  Tiny
          "keep-warm" matmuls after each exp stop the HAM clock-gate from
          re-throttling the PE between bursts.

Build requirements discovered the hard way: the TRN2 ISA has ONE semaphore
wait slot per instruction, so the program must be built with bacc.Bacc and
nc.compile() (its generate_event_semaphores / move_matmul_waits_to_ldweights
passes legalize Tile's multi-wait instructions).  Input DMAs are split across
both HW-DGE rings (SP + ACT) and ordered so the projection's inputs land
first.
"""

import numpy as np
from contextlib import ExitStack

import concourse.bass as bass
import concourse.bacc as bacc
import concourse.tile as tile
from concourse import mybir
from concourse.bass_utils import run_bass_kernel_spmd

TD, TE, B = 128, 128, 8
E, H, D = 512, 1024, 256
P = 128
CE = E // P          # 4 e-chunks
CH = H // P          # 8 h-chunks
TB = 16              # t-block size
NBLK = TD // TB      # 8 blocks
ACT_BLOCKS = (0, 4)     # blocks on the fused ACT path
POOL_CE = 3             # chunks 0..POOL_CE-1 of DVE-path blocks multiply on GPSIMD
POOL_REDUCE = frozenset({(5, 0), (7, 0), (7, 1)})  # (blk, ce) reduces done as GPSIMD trees

_F32 = mybir.dt.float32
_CACHE = {}


def _kernel_body(ctx, tc, out_ap, wt_ap, dtr_ap, et_ap, ei_ap):
    nc = tc.nc
    AF = mybir.ActivationFunctionType

    singles = ctx.enter_context(tc.tile_pool(name="singles", bufs=1))
    p_pool = ctx.enter_context(tc.tile_pool(name="p", bufs=4))
    tr_pool = ctx.enter_context(tc.tile_pool(name="tr", bufs=2))
    psum_pool = ctx.enter_context(tc.tile_pool(name="psum", bufs=2, space="PSUM"))
    psum_w = ctx.enter_context(tc.tile_pool(name="psum_w", bufs=2, space="PSUM"))
    psum_o = ctx.enter_context(tc.tile_pool(name="psum_o", bufs=1, space="PSUM"))

    # ---- load inputs (natural-layout DMAs; transposes were done on host)
    # spread input DMAs over both HW-DGE rings (SP and ACT) for 2x bandwidth,
    # ordered so the projection's critical inputs (dt + wt slab 0) land first
    dt_sb = singles.tile([P, CH, TD], _F32)  # D.T chunks
    dt_r = dtr_ap.rearrange("(c p) t -> p c t", p=P)
    nc.scalar.dma_start(out=dt_sb[:, 0:CH // 2, :], in_=dt_r[:, 0:CH // 2, :])
    wt_sb = singles.tile([P, CH, CE, P], _F32)  # [hp, hc, ce, e_local]
    wt_r = wt_ap.rearrange("ce (c p) m -> p c ce m", p=P)
    nc.sync.dma_start(out=wt_sb[:, :, 0, :], in_=wt_r[:, :, 0, :])
    nc.scalar.dma_start(out=dt_sb[:, CH // 2:, :], in_=dt_r[:, CH // 2:, :])
    et_sb = singles.tile([P, CE, TE], _F32)  # enc.T chunks: [e_local, ce, s]
    nc.sync.dma_start(out=et_sb[:], in_=et_ap.rearrange("(c p) s -> p c s", p=P))
    for ce in range(1, CE):
        eng = nc.sync if ce % 2 == 0 else nc.scalar
        eng.dma_start(out=wt_sb[:, :, ce, :], in_=wt_r[:, :, ce, :])
    ei_sb = singles.tile([P, D], _F32)       # enc_in natural [s, d]
    nc.scalar.dma_start(out=ei_sb[:], in_=ei_ap)

    # per-block statistics live in static tiles (never recycled, so slot
    # reuse never attaches extra semaphore waits)
    den_all = singles.tile([P, NBLK, CE, TB], _F32)
    r_all = singles.tile([P, NBLK, CE, TB], _F32)

    # ---- projection: dec_T[e, t] = sum_h W.T[h, e] * D.T[h, t]
    dec_sb = singles.tile([P, CE, TD], _F32)  # [e_local, ce, t]
    for ce in range(CE):
        dps = psum_pool.tile([P, TD], _F32)
        # ce0's first t-block is the whole pipeline's gate: compute those 16
        # columns in a short N=16 chain first so downstream engines start ~3us
        # earlier, then the rest
        tranges = (((0, TB), (TB, 2 * TB), (2 * TB, TD)) if ce == 0
                   else ((0, 2 * TB), (2 * TB, TD)))
        for lo, hi in tranges:
            for c in range(CH):
                nc.tensor.matmul(
                    dps[:, lo:hi],
                    lhsT=wt_sb[:, c, ce, :],
                    rhs=dt_sb[:, c, lo:hi],
                    start=(c == 0),
                    stop=(c == CH - 1),
                )
            nc.vector.tensor_copy(dec_sb[:, ce, lo:hi], dps[:, lo:hi])

    # ---- softmax + weighted e-sums, pipelined over t-blocks
    wsum_sb = singles.tile([P, TD], _F32)    # wsum_T[s, t], filled per block
    # keep-warm pokes: tiny matmuls spread across each block keep the PE HAM
    # clock-gate from re-throttling during the ~7us inter-burst waits
    psum_k = ctx.enter_context(tc.tile_pool(name="psum_k", bufs=1, space="PSUM"))
    warm_ps = psum_k.tile([1, NBLK * CE + 8], _F32)
    _warm = [0]

    def pe_warm(col):
        k = _warm[0]
        _warm[0] += 1
        nc.tensor.matmul(warm_ps[0:1, k:k + 1], lhsT=col, rhs=col,
                         start=True, stop=True)

    for blk in range(NBLK):
        t0 = blk * TB

        p_t = p_pool.tile([P, CE, TB, TE], _F32)   # p[e_local, ce, t_local, s]
        den = den_all[:, blk, :, :]

        if blk in ACT_BLOCKS:
            # fused: p = exp(enc_T * dec_col); denominators via DVE reduce
            for ce in range(CE):
                for tl in range(TB):
                    t = t0 + tl
                    nc.scalar.activation(
                        out=p_t[:, ce, tl, :],
                        in_=et_sb[:, ce, :],
                        func=AF.Exp,
                        scale=dec_sb[:, ce, t:t + 1],
                    )
                if (blk, ce) in POOL_REDUCE:
                    # tail blocks: tree-reduce on the (by now idle) GPSIMD
                    # engine so the last reduces don't pile up on DVE
                    tmp = tr_pool.tile([P, TB, TE // 2], _F32)
                    half = TE // 2
                    nc.gpsimd.tensor_add(
                        tmp[:, :, 0:half],
                        p_t[:, ce, :, 0:half], p_t[:, ce, :, half:TE])
                    w = half // 2
                    while w >= 2:
                        nc.gpsimd.tensor_add(
                            tmp[:, :, 0:w], tmp[:, :, 0:w], tmp[:, :, w:2 * w])
                        w //= 2
                    nc.gpsimd.tensor_add(
                        den[:, ce, :], tmp[:, :, 0:1], tmp[:, :, 1:2])
                else:
                    nc.vector.reduce_sum(
                        out=den[:, ce, :], in_=p_t[:, ce, :, :],
                        axis=mybir.AxisListType.X,
                    )
        else:
            sc_t = p_t
            for ce in range(CE):
                # scores[e,(t,s)] = dec_T[e,t] * enc_T[e,s] via stride-0 APs
                dslice = dec_sb[:, ce, t0:t0 + TB]
                dec_b = bass.AP(
                    tensor=dslice.tensor, offset=dslice.offset,
                    ap=[dslice.ap[0], dslice.ap[1], [0, TE]],
                )
                eslice = et_sb[:, ce, :]
                enc_b = bass.AP(
                    tensor=eslice.tensor, offset=eslice.offset,
                    ap=[eslice.ap[0], [0, TB], eslice.ap[1]],
                )
                if ce < (2 if blk == 7 else POOL_CE):
                    nc.gpsimd.tensor_mul(sc_t[:, ce, :, :], dec_b, enc_b)
                else:
                    nc.vector.tensor_mul(sc_t[:, ce, :, :], dec_b, enc_b)
                nc.scalar.activation(
                    out=p_t[:, ce, :, :], in_=p_t[:, ce, :, :], func=AF.Exp,
                )
                pe_warm(p_t[:, ce, 0, 0:1])
                if (blk, ce) in POOL_REDUCE:
                    # tail blocks: tree-reduce on the (by now idle) GPSIMD
                    # engine so the last reduces don't pile up on DVE
                    tmp = tr_pool.tile([P, TB, TE // 2], _F32)
                    half = TE // 2
                    nc.gpsimd.tensor_add(
                        tmp[:, :, 0:half],
                        p_t[:, ce, :, 0:half], p_t[:, ce, :, half:TE])
                    w = half // 2
                    while w >= 2:
                        nc.gpsimd.tensor_add(
                            tmp[:, :, 0:w], tmp[:, :, 0:w], tmp[:, :, w:2 * w])
                        w //= 2
                    nc.gpsimd.tensor_add(
                        den[:, ce, :], tmp[:, :, 0:1], tmp[:, :, 1:2])
                else:
                    nc.vector.reduce_sum(
                        out=den[:, ce, :], in_=p_t[:, ce, :, :],
                        axis=mybir.AxisListType.X,
                    )

        r_t = r_all[:, blk, :, :]
        nc.vector.reciprocal(out=r_t, in_=den)
        r2 = r_t

        # wsum_T[:, t] = sum_e p[e, t, :].T @ r[e, t] (4 accumulating N=1 MMs)
        wps = psum_w.tile([P, TB], _F32)
        for tl in range(TB):
            for ce in range(CE):
                nc.tensor.matmul(
                    wps[:, tl:tl + 1],
                    lhsT=p_t[:, ce, tl, :],
                    rhs=r2[:, ce, tl:tl + 1],
                    start=(ce == 0),
                    stop=(ce == CE - 1),
                )
        nc.vector.tensor_copy(wsum_sb[:, t0:t0 + TB], wps[:])

    # ---- final: out[t, d] = sum_s wsum_T[s, t] * enc_in[s, d]
    # done per pair of t-blocks (M=32, legal PSUM partition offsets) so most
    # of the final matmul/copy/store retires before the last block finishes
    out_ps = psum_o.tile([P, D], _F32)
    out_sb = singles.tile([P, D], _F32)
    for q in range(NBLK // 2):
        t0 = q * 2 * TB
        nc.tensor.matmul(out_ps[t0:t0 + 2 * TB, :],
                         lhsT=wsum_sb[:, t0:t0 + 2 * TB], rhs=ei_sb[:],
                         start=True, stop=True, tile_position=(0, t0))
        nc.vector.tensor_copy(out_sb[t0:t0 + 2 * TB, :], out_ps[t0:t0 + 2 * TB, :])
        nc.sync.dma_start(out=out_ap[t0:t0 + 2 * TB, :], in_=out_sb[t0:t0 + 2 * TB, :])


def build_program():
    if "nc" in _CACHE:
        return _CACHE["nc"]
    nc = bacc.Bacc("TRN2", target_bir_lowering=False, debug=False, num_devices=B)
    wt = nc.dram_tensor("wt", [CE, H, P], _F32, kind="ExternalInput").ap()
    dtr = nc.dram_tensor("dtr", [H, TD], _F32, kind="ExternalInput").ap()
    et = nc.dram_tensor("et", [E, TE], _F32, kind="ExternalInput").ap()
    ei = nc.dram_tensor("ei", [TE, D], _F32, kind="ExternalInput").ap()
    out = nc.dram_tensor("out", [TD, D], _F32, kind="ExternalOutput").ap()
    with tile.TileContext(nc) as tc:
        with ExitStack() as ctx:
            _kernel_body(ctx, tc, out, wt, dtr, et, ei)
    nc.compile()
    _CACHE["nc"] = nc
    return nc


def make_in_maps(encoder_inputs, encoder_states, decoder_states, W):
    wtt = W.T  # (H, E)
    wt_np = np.ascontiguousarray(
        wtt.reshape(H, CE, P).transpose(1, 0, 2))  # (CE, H, 128), shared
    in_maps = []
    for b in range(B):
        in_maps.append({
            "wt": wt_np,
            "dtr": np.ascontiguousarray(decoder_states[:, b, :].T),  # (H, TD)
            "et": np.ascontiguousarray(encoder_states[:, b, :].T),   # (E, TE)
            "ei": np.ascontiguousarray(encoder_inputs[:, b, :]),     # (TE, D)
        })
    return in_maps


def run_on_hw(in_maps, **kwargs):
    nc = build_program()
    return run_bass_kernel_spmd(nc, in_maps, list(range(B)), **kwargs)


def kernel(**inputs):
    encoder_inputs = np.asarray(inputs["encoder_inputs"], dtype=np.float32)
    encoder_states = np.asarray(inputs["encoder_states"], dtype=np.float32)
    decoder_states = np.asarray(inputs["decoder_states"], dtype=np.float32)
    W = np.asarray(inputs["W"], dtype=np.float32)
    in_maps = make_in_maps(encoder_inputs, encoder_states, decoder_states, W)
    res = run_on_hw(in_maps)
    out = np.stack([res.results[b]["out"] for b in range(B)], axis=1)
    return np.ascontiguousarray(out.astype(np.float32))

